# revision 13
# baseline (speedup 1.0000x reference)
"""Causal self-attention TRN2 kernel, v2.

Full module: x[4,2048,1024] @ W_qkv[1024,3072] -> heads(16, d=64) causal attn
-> @ W_proj[1024,1024].

Sharding: 8 cores = 4 batches x 2 head-groups (8 heads each), tensor-parallel
over heads. Each core computes q/k/v for its 8 heads, causal attention, and a
partial projection (row-sharded W_proj). The two partials per batch are summed
on the host (no on-device collectives).

v2 changes vs baseline:
  - weights (wq/wk/wv/wp) DMA'd once per pass as [128,512]/[128,1024] tiles
    and kept resident (baseline re-loaded wq/wk every 512-token block: 4x).
  - x streamed in 512-token quarters with a 2-block pool (16 bufs).
  - attention de-paired: one [128,512] PSUM bank per (head, k-tile) score
    tile instead of [128,1024] per head-pair; frees banks for dedicated
    per-phase PSUM pools (s:3, y:2, qkv:2, proj:1) so QKV(n+1)/attn(n)/
    proj(n) can overlap without false pool serialization.
  - causal masking via ONE gpsimd affine_select per diagonal tile (zeroes
    the dead columns AND the upper triangle in place); kills the zero-DMAs
    and DVE mask multiplies of the baseline.
  - softmax division: DVE reciprocal -> gpsimd partition_broadcast -> DVE
    multiply (baseline used K=1 PE matmuls to broadcast).
  - PSUM->SBUF copies split across engines: q/k/v+div on DVE, proj-out on
    gpsimd; ACT does exp only.
"""

import numpy as np
from contextlib import ExitStack

import concourse.bass as bass
import concourse.tile as tile
from concourse import mybir, bacc
from concourse.bass_utils import run_bass_kernel_spmd

F32 = mybir.dt.float32
F32R = mybir.dt.float32r
EXP = mybir.ActivationFunctionType.Exp
GE = mybir.AluOpType.is_ge

B, T, C, H, D = 4, 2048, 1024, 16, 64
NCORES = 8
GROUPS = 2            # head groups (tensor-parallel dimension)
HPC = H // GROUPS     # heads per core = 8
FPC = HPC * D         # features per core = 512
SCALE = 1.0 / np.sqrt(D)


def build_nc(T=T, C=C, HPC=HPC, repeat=1, serialize_reps=False):
    FPC = HPC * D
    NC = C // 128     # contraction chunks over C = 8
    NT = T // 128     # token tiles (also k-tiles) = 16
    NQ = T // 512     # query chunks (pipeline blocks) = 4
    NF = FPC // 128   # feature tiles = head pairs = 4

    nc = bacc.Bacc("TRN2", debug=False)
    xT_d = nc.dram_tensor("xT", [C, T], F32R, kind="ExternalInput").ap()
    wq_d = nc.dram_tensor("wq", [C, FPC], F32R, kind="ExternalInput").ap()
    wk_d = nc.dram_tensor("wk", [C, FPC], F32R, kind="ExternalInput").ap()
    wv_d = nc.dram_tensor("wv", [C, FPC], F32R, kind="ExternalInput").ap()
    wp_d = nc.dram_tensor("wp", [FPC, C], F32R, kind="ExternalInput").ap()
    out_d = nc.dram_tensor("out", [T, C], F32, kind="ExternalOutput").ap()

    with tile.TileContext(nc) as tc, ExitStack() as ctx:
        p_w = ctx.enter_context(tc.tile_pool(name="p_w", bufs=NC))
        p_wp = ctx.enter_context(tc.tile_pool(name="p_wp", bufs=NF))
        p_xq = ctx.enter_context(tc.tile_pool(name="p_xq", bufs=13))
        p_kt = ctx.enter_context(tc.tile_pool(name="p_kt", bufs=NF))
        p_v65 = ctx.enter_context(tc.tile_pool(name="p_v65", bufs=NT))
        p_qtq = ctx.enter_context(tc.tile_pool(name="p_qtq", bufs=2 * NF))
        p_ytq = ctx.enter_context(tc.tile_pool(name="p_ytq", bufs=2 * NF))
        p_pt = ctx.enter_context(tc.tile_pool(name="p_pt", bufs=2))
        p_rec = ctx.enter_context(tc.tile_pool(name="p_rec", bufs=1))
        p_ybt = ctx.enter_context(tc.tile_pool(name="p_ybt", bufs=1))
        p_osb = ctx.enter_context(tc.tile_pool(name="p_osb", bufs=2))
        # 8 PSUM banks: paired scores 2x2 + y-accum 2 + qkv/proj shared 2
        ps_s = ctx.enter_context(tc.tile_pool(name="ps_s", bufs=2, space="PSUM"))
        ps_y = ctx.enter_context(tc.tile_pool(name="ps_y", bufs=2, space="PSUM"))
        ps_k = ctx.enter_context(tc.tile_pool(name="ps_k", bufs=2, space="PSUM"))

        def emit_once(rep):
            serdep = serialize_reps and rep > 0
            xq = {}

            def dma_xq(n):
                for c in range(NC):
                    t_ = p_xq.tile([128, 512], F32R, tag="xq", name=f"xq{c}_{n}")
                    src = xT_d[c * 128:(c + 1) * 128, n * 512:(n + 1) * 512]
                    if serdep:
                        # timing mode: token-block n of a repeat's input
                        # sources from the previous repeat's output rows for
                        # the same tokens — the real dataflow of stacked
                        # layers (block-pipelined; weights prefetchable).
                        src = out_d[(4 * n + c % 4) * 128:
                                    (4 * n + c % 4 + 1) * 128,
                                    0:512].bitcast(F32R)
                    nc.sync.dma_start(out=t_[:], in_=src)
                    xq[(c, n)] = t_

            dma_xq(0)
            # --- weights: resident for the whole pass ---
            wq_sb, wk_sb, wv_sb = [], [], []
            for store, w_d, wtag in ((wq_sb, wq_d, "wq"), (wk_sb, wk_d, "wk"),
                                     (wv_sb, wv_d, "wv")):
                for c in range(NC):
                    wt = p_w.tile([128, FPC], F32R, tag=wtag, name=f"{wtag}{c}")
                    nc.sync.dma_start(out=wt[:], in_=w_d[c * 128:(c + 1) * 128, :])
                    store.append(wt)
            wp_sb = []
            for cf in range(NF):
                wt = p_wp.tile([128, C], F32R, tag="wp", name=f"wp{cf}")
                nc.sync.dma_start(out=wt[:], in_=wp_d[cf * 128:(cf + 1) * 128, :])
                wp_sb.append(wt)

            kt_ = [p_kt.tile([128, T], F32R, tag="kt", name=f"kt{i}")
                   for i in range(NF)]
            v65 = [p_v65.tile([128, HPC, 65], F32R, tag="v65", name=f"v65_{i}")
                   for i in range(NT)]
            qtq = {}   # (f, qc) -> [128, 512] query quarter (f32r, transposed)
            ytq = {}   # (f, qc) -> [128, 512] attention-out quarter

            def q_group(n, f, isq):
                store = wq_sb if isq else wk_sb
                ps = ps_k.tile([128, 512], F32, tag="qk", name=f"qk{n}_{f}")
                for c in range(NC):
                    nc.tensor.matmul(
                        ps[:], store[c][:, f * 128:(f + 1) * 128], xq[(c, n)][:],
                        start=(c == 0), stop=(c == NC - 1))
                if isq:
                    dst = p_qtq.tile([128, 512], F32R, tag="qt",
                                     name=f"qtq{f}_{n}")
                    qtq[(f, n)] = dst
                    nc.vector.tensor_copy(out=dst[:], in_=ps[:])
                else:
                    nc.vector.tensor_copy(
                        out=kt_[f][:, n * 512:(n + 1) * 512], in_=ps[:])

            def v_group(n, t):
                ps = ps_k.tile([128, 512], F32, tag="qk", name=f"v{t}")
                for c in range(NC):
                    nc.tensor.matmul(
                        ps[:], xq[(c, n)][:, (t % 4) * 128:(t % 4 + 1) * 128],
                        wv_sb[c][:], start=(c == 0), stop=(c == NC - 1))
                nc.vector.tensor_copy(
                    out=v65[t][:, :, 0:64],
                    in_=ps[:].rearrange("p (h d) -> p h d", h=HPC))
                nc.vector.memset(v65[t][:, :, 64:65].bitcast(F32), 1.0)

            def attention_head(qc, hp, sub):
                """head h = 2*hp + sub; scoresT/exp/PV for q-chunk qc.

                Emitted at high priority: when the PE has both an attention
                matmul and QKV/proj filler ready, prefer the attention matmul
                so the ACT exp stream (the pacer of late blocks) never
                starves; QKV/proj then soak up the ACT-paced stalls.
                """
                nk = 4 * qc + 4
                h = 2 * hp + sub
                rsl = slice(64 * sub, 64 * sub + 64)
                qtile = qtq[(hp, qc)]
                y_ps = ps_y.tile([65, 512], F32, tag="y", name=f"y{qc}_{h}")
                # k-tiles in pairs sharing a [128,1024] 2-bank PSUM tile so
                # off-diagonal pairs need ONE exp instruction (ACT's fixed
                # per-instruction overhead is ~15% of a 512-col exp)
                for kp in range(nk // 2):
                    s_ps = ps_s.tile([128, 1024], F32, tag="s")
                    pt = p_pt.tile([128, 1024], F32R, tag="pt")
                    dpair = []
                    for half in range(2):
                        kt = 2 * kp + half
                        d = kt - 4 * qc   # >=0: diagonal tile, cols<128d dead
                        lo = 128 * d if d > 0 else 0
                        cs = slice(512 * half + lo, 512 * half + 512)
                        dpair.append((kt, d, lo, cs))
                        nc.tensor.matmul(
                            s_ps[:, cs], kt_[hp][rsl, kt * 128:(kt + 1) * 128],
                            qtile[rsl, lo:512],
                            start=True, stop=True, tile_position=(64 * sub, 0))
                    if dpair[0][1] < 0 and dpair[1][1] < 0:
                        nc.scalar.activation(
                            out=pt[:], in_=s_ps[:], func=EXP,
                            scale=float(SCALE))
                    else:
                        for kt, d, lo, cs in dpair:
                            nc.scalar.activation(
                                out=pt[:, cs], in_=s_ps[:, cs], func=EXP,
                                scale=float(SCALE))
                    for kt, d, lo, cs in dpair:
                        if d >= 0:
                            # zero the upper triangle of the diagonal 128x128
                            # block in place: keep pt[p,j] iff j - lo - p >= 0
                            tri = slice(512 * (kt & 1) + lo,
                                        512 * (kt & 1) + lo + 128)
                            nc.gpsimd.affine_select(
                                out=pt[:, tri], in_=pt[:, tri],
                                pattern=[[1, 128]], compare_op=GE, fill=0.0,
                                base=0, channel_multiplier=-1)
                        nc.tensor.matmul(
                            y_ps[:, lo:512], v65[kt][:, h, :], pt[:, cs],
                            start=(kt == 0), stop=(kt == nk - 1))

                # softmax division. HW quirks force every engine-op AP here to
                # start at partition 0 (gpsimd partition_broadcast reads
                # garbage from offset APs; DVE rejects non-multiple-of-32
                # offsets), so: DMA the denominator row (psum row 64) to
                # partition 0 of an SBUF tile, reciprocal in place, gpsimd-
                # broadcast it over rows 0-63, and multiply lane-aligned.
                rec = p_rec.tile([65, 512], F32R, tag="rec")
                with nc.allow_low_precision("f32r softmax denom reciprocal"):
                    nc.vector.reciprocal(out=rec[64:65, :], in_=y_ps[64:65, :])
                nc.sync.dma_start(out=rec[0:1, :], in_=rec[64:65, :])
                nc.gpsimd.partition_broadcast(rec[0:64, :], rec[0:1, :],
                                              channels=64)
                if sub == 0:
                    ytile = p_ytq.tile([128, 512], F32R, tag="yt",
                                       name=f"ytq{hp}_{qc}")
                    ytq[(hp, qc)] = ytile
                    nc.vector.tensor_mul(ytile[0:64, :], y_ps[0:64, :], rec[0:64, :])
                else:
                    # odd head lands on partitions 64..127 of the pair tile:
                    # DVE is lane-locked, so stage then DMA-shift
                    ybt = p_ybt.tile([64, 512], F32R, tag="ybt")
                    nc.vector.tensor_mul(ybt[:], y_ps[0:64, :], rec[0:64, :])
                    nc.sync.dma_start(out=ytq[(hp, qc)][64:128, :], in_=ybt[:])

            def proj_t(qc, t):
                tloc = (t - 4 * qc) * 128
                osb = p_osb.tile([128, C], F32, tag="osb", name=f"osb{t}")
                for nn in range(2):
                    pj = ps_k.tile([128, 512], F32, tag="qk", name=f"pj{t}_{nn}")
                    for cf in range(NF):
                        nc.tensor.matmul(
                            pj[:],
                            ytq[(cf, qc)][:, tloc:tloc + 128],
                            wp_sb[cf][:, nn * 512:(nn + 1) * 512],
                            start=(cf == 0), stop=(cf == NF - 1))
                    nc.vector.tensor_copy(
                        out=osb[:, nn * 512:(nn + 1) * 512], in_=pj[:])
                nc.sync.dma_start(
                    out=out_d[t * 128:(t + 1) * 128, :], in_=osb[:])

            # proj(qc) is emitted after QKV(qc+1) so in the shared qkv/proj
            # PSUM pool the slot order is [qkv(qc+1)..., pj(qc)...]: QKV(qc+1)
            # overlaps attention(qc), and proj(qc) becomes attention(qc+1)'s
            # PE filler. Attention itself is high priority so the exp stream
            # (the pacer of late blocks) is fed first.
            for qc in range(NQ):
                if qc + 1 < NQ:
                    dma_xq(qc + 1)
                for f in range(NF):
                    q_group(qc, f, True)
                for f in range(NF):
                    q_group(qc, f, False)
                for t in range(4 * qc, 4 * qc + 4):
                    v_group(qc, t)
                if qc > 0:
                    for t in range(4 * (qc - 1), 4 * (qc - 1) + 4):
                        proj_t(qc - 1, t)
                for hp in range(NF):
                    for sub in range(2):
                        with tc.high_priority():
                            attention_head(qc, hp, sub)
            for t in range(4 * (NQ - 1), 4 * (NQ - 1) + 4):
                proj_t(NQ - 1, t)

        for _rep in range(repeat):
            emit_once(_rep)
    nc.finalize()
    return nc


def make_in_maps(x, W_qkv, W_proj):
    """Host-side sharding of full inputs into per-core input maps."""
    x = np.asarray(x, dtype=np.float32)
    W_qkv = np.asarray(W_qkv, dtype=np.float32)
    W_proj = np.asarray(W_proj, dtype=np.float32)
    in_maps = []
    for core in range(NCORES):
        b, g = core // GROUPS, core % GROUPS
        in_maps.append({
            "xT": np.ascontiguousarray(x[b].T),
            "wq": np.ascontiguousarray(W_qkv[:, g * FPC:(g + 1) * FPC]),
            "wk": np.ascontiguousarray(W_qkv[:, C + g * FPC:C + (g + 1) * FPC]),
            "wv": np.ascontiguousarray(W_qkv[:, 2 * C + g * FPC:2 * C + (g + 1) * FPC]),
            "wp": np.ascontiguousarray(W_proj[g * FPC:(g + 1) * FPC, :]),
        })
    return in_maps


_CACHE = {}


def _get_nc():
    if "nc" not in _CACHE:
        _CACHE["nc"] = build_nc()
    return _CACHE["nc"]


def run_cores(in_maps):
    res = run_bass_kernel_spmd(_get_nc(), in_maps, list(range(NCORES)))
    return res.results


def kernel(x, W_qkv, W_proj):
    results = run_cores(make_in_maps(x, W_qkv, W_proj))
    out = np.empty((B, T, C), dtype=np.float32)
    for b in range(B):
        out[b] = results[GROUPS * b]["out"]
        for g in range(1, GROUPS):
            out[b] += results[GROUPS * b + g]["out"]
    return out
